# revision 13
# baseline (speedup 1.0000x reference)
"""Distributed GCN (3x GCNConv + linear head) on 8 TRN2 NeuronCores.

Strategy (graph/data parallel, per sharding hint):
  - Nodes block-sharded across 8 cores (5000 real rows each, padded to 5120).
  - Weights replicated; per-layer: p = H @ W computed locally per 128-node
    window (node-major pc tiles kept in SBUF as `plocal`), cast to fp16 and
    DMA'd into two staging buffers agin_A (local rows 0..2559) / agin_B
    (2560..5119). Two AllGathers (A fires as soon as windows 0..19 are done,
    overlapping the rest of the previous layer's aggregation) build two
    shared tables pfull_A/pfull_B [8*2560, 128] fp16 in DRAM.
  - Edges assigned to the core owning dst; self-loop terms are NOT edges:
    they are added per window with one extra matmul against a host-built
    diagonal S (diag(dinv^2)) using the SBUF-resident plocal tiles.
  - Remaining edges are grouped by (dst PAIR-window of 256 nodes, A/B table)
    and padded to 128-edge tiles. Per-edge norm dinv[src]*dinv[dst] is folded
    into host-built one-hot scatter tiles S [slot, dst-in-pair] fp16 (256
    wide, so each gathered tile needs a single wide matmul).
  - Gather: batched nc.gpsimd.dma_gather (int16 idxs < 20480) across 4 SWDGE
    queues (round-robin; each queue's descriptor generation runs on its own
    Q7 core pair, so up to 4 generations run concurrently) pulls message rows
    M [128 slots, 128 feat] fp16 from pfull_{A,B}; aggregation is
    PSUM[f, 256] += M^T @ S on the PE, flushed with Relu+bias on ACT into the
    next layer's H^T. The next layer's p-matmul for window w is emitted right
    after window w's flush.
  - Head: out = H3 @ lin_w + lin_b via PE + transpose, one [5120] f32 per
    core, host concatenates and trims padding.

Self-contained: hardcodes the problem shapes; all host-side prep derives
from the runtime edge_index only (index bookkeeping + degree).
"""

import os
from contextlib import ExitStack
from dataclasses import dataclass, field

import numpy as np

import concourse.bacc as bacc
import concourse.bass as bass
import concourse.mybir as mybir
import concourse.tile as tile
from concourse.bass_utils import run_bass_kernel_spmd

F32 = mybir.dt.float32
F16 = mybir.dt.float16
I16 = mybir.dt.int16
AF = mybir.ActivationFunctionType
ALU = mybir.AluOpType

D = 128  # feature dim (in = hid = 128)
WIN = 128  # dst nodes per flush window
GW = 256  # dst nodes per aggregation (pair) group / S width
NC = 8  # cores


@dataclass
class Cfg:
    n: int = 40000
    e: int = 640000
    shard: int = 5000  # real nodes per core
    msg_dtype: object = F16

    @property
    def spad(self):  # padded shard
        return ((self.shard + GW - 1) // GW) * GW

    @property
    def nwin(self):
        return self.spad // WIN

    @property
    def npair(self):
        return self.spad // GW

    @property
    def hrowsA(self):  # local rows in table A (int16-max biased)
        return max(WIN, min(32768 // NC, (self.spad * 4 // 5) // WIN * WIN))

    @property
    def hrowsB(self):
        return self.spad - self.hrowsA

    @property
    def hrows(self):  # per-half local rows, indexed
        return (self.hrowsA, self.hrowsB)


@dataclass
class Plan:
    """Per-call schedule shared by all cores (static SPMD program)."""

    caps: np.ndarray  # [npair, 2] tiles per (pair, half), max over cores
    tot: int  # total tiles per layer
    chunks: list = field(default_factory=list)
    # chunks: one per pair group g:
    #  {"g": g, "nt": {h: ntiles}, "t0": {h: first-global-tile},
    #   "tiles": [(h, off_in_half_buf, global_tile), ...]}


def build_plan(caps: np.ndarray, cfg: Cfg) -> Plan:
    plan = Plan(caps=caps, tot=int(caps.sum()))
    t = 0
    for g in range(cfg.npair):
        ch = {"g": g, "nt": {}, "t0": {}, "tiles": []}
        for h in (0, 1):
            ch["t0"][h] = t
            off = 0
            for _ in range(int(caps[g, h])):
                ch["tiles"].append((h, off, t))
                off += 1
                t += 1
            ch["nt"][h] = off
        plan.chunks.append(ch)
    assert t == plan.tot
    return plan


def preprocess(edge_index: np.ndarray, cfg: Cfg):
    """Host-side index prep. Returns (plan, per_core dict arrays, dinv)."""
    n, shard, spad = cfg.n, cfg.shard, cfg.spad
    hrA, hrB = cfg.hrows
    npair, nwin = cfg.npair, cfg.nwin
    src = edge_index[0].astype(np.int64)
    dst = edge_index[1].astype(np.int64)
    deg = 1.0 + np.bincount(dst, minlength=n).astype(np.float64)
    dinv = (1.0 / np.sqrt(deg)).astype(np.float32)

    allnorm = (dinv[src] * dinv[dst]).astype(np.float32)

    core = dst // shard
    dloc = dst % shard
    g = dloc // GW
    dwin = (dloc % GW).astype(np.float32)
    sc = src // shard
    sloc = src % shard
    h = (sloc >= hrA).astype(np.int64)  # 0 = table A, 1 = table B
    idx = np.where(
        h == 0, sc * hrA + sloc, sc * hrB + (sloc - hrA)
    ).astype(np.int16)
    assert NC * hrA <= 32768 and NC * hrB <= 32768

    key = (core * npair + g) * 2 + h
    cnt = np.bincount(key, minlength=NC * npair * 2).reshape(NC, npair, 2)
    caps = np.ceil(cnt.max(axis=0) / 128.0).astype(np.int64)  # [npair, 2]
    caps = np.maximum(caps, 1)
    plan = build_plan(caps, cfg)
    tot = plan.tot

    order = np.lexsort((h, g, core))
    osrcidx = idx[order]
    odwin = dwin[order]
    onorm = allnorm[order]
    okey = key[order]
    starts = np.zeros(NC * npair * 2 + 1, dtype=np.int64)
    np.cumsum(np.bincount(okey, minlength=NC * npair * 2), out=starts[1:])

    # first tile of each (pair, half) group in the tile stream
    gslot = np.zeros((npair, 2), dtype=np.int64)
    for ch in plan.chunks:
        firsts = {}
        for hh, _off, gt in ch["tiles"]:
            if hh not in firsts:
                firsts[hh] = gt
        for hh, gt in firsts.items():
            gslot[ch["g"], hh] = gt

    per_core = []
    for c in range(NC):
        gi = np.zeros(tot * 128, dtype=np.int16)
        dl = np.zeros(tot * 128, dtype=np.float32)
        nv = np.zeros(tot * 128, dtype=np.float32)
        for gv in range(npair):
            for hh in (0, 1):
                k = (c * npair + gv) * 2 + hh
                s, e_ = starts[k], starts[k + 1]
                m = e_ - s
                if m == 0:
                    continue
                base = gslot[gv, hh] * 128
                assert m <= caps[gv, hh] * 128
                gi[base : base + m] = osrcidx[s:e_]
                dl[base : base + m] = odwin[s:e_]
                nv[base : base + m] = onorm[s:e_]
        # gather idx layout: idx i -> [i%16 (+16k replicas), i//16]
        gi16 = gi.reshape(tot * 8, 16).T  # [16, tot*8]
        gi128 = np.tile(gi16, (8, 1)).copy()  # [128, tot*8]
        # host-prebuilt scatter one-hots S [tot*128, GW] -> partition-major
        sf = np.zeros((tot * 128, GW), np.float16)
        sf[np.arange(tot * 128), dl.astype(np.int64)] = nv.astype(np.float16)
        sflat = np.ascontiguousarray(
            sf.reshape(tot, 128, GW).transpose(1, 0, 2).reshape(128, tot * GW)
        )
        # self-loop diagonal S per window: diag(dinv^2) over local rows
        dg = np.zeros((128, nwin * 128), np.float16)
        for wv in range(nwin):
            rows = np.arange(wv * 128, (wv + 1) * 128) + c * shard
            val = np.where(
                np.arange(wv * 128, (wv + 1) * 128) < shard,
                (dinv[np.minimum(rows, n - 1)] ** 2),
                0.0,
            ).astype(np.float16)
            dg[np.arange(128), wv * 128 + np.arange(128)] = val
        per_core.append({"gidx": gi128, "sflat": sflat, "diag": dg})
    return plan, per_core, dinv


def emulate(x, edge_index, Ws, bs, lin_w, lin_b, cfg: Cfg, fp16=True):
    """Numpy emulation of the exact device dataflow (for validation)."""
    plan, per_core, dinv = preprocess(edge_index, cfg)
    spad, nwin = cfg.spad, cfg.nwin
    hrA, hrB = cfg.hrows
    md = np.float16 if fp16 else np.float32
    H = []
    for c in range(NC):
        xs = x[c * cfg.shard : (c + 1) * cfg.shard]
        H.append(
            np.concatenate([xs, np.zeros((spad - cfg.shard, D), np.float32)]).T.copy()
        )
    for l in range(3):
        W, b = Ws[l], bs[l]
        pf = [np.zeros((NC * hrA, D), md), np.zeros((NC * hrB, D), md)]
        plocal = []
        for c in range(NC):
            p = (H[c].T.astype(np.float32) @ W).astype(md)
            pf[0][c * hrA : (c + 1) * hrA] = p[:hrA]
            pf[1][c * hrB : (c + 1) * hrB] = p[hrA:]
            plocal.append(p)
        Hn = []
        for c in range(NC):
            pc = per_core[c]
            HT = np.zeros((D, spad), np.float32)
            for ch in plan.chunks:
                gv = ch["g"]
                acc = np.zeros((D, GW), np.float32)
                for hh, _off, gt in ch["tiles"]:
                    ii = pc["gidx"][:16, gt * 8 : gt * 8 + 8].T.reshape(-1)
                    M = pf[hh][ii.astype(np.int64)]
                    S = pc["sflat"][:, gt * GW : (gt + 1) * GW]
                    acc += M.astype(np.float32).T @ S.astype(np.float32)
                for j in range(2):
                    wv = 2 * gv + j
                    Mw = plocal[c][wv * 128 : (wv + 1) * 128]
                    Sd = pc["diag"][:, wv * 128 : (wv + 1) * 128]
                    acc[:, j * 128 : (j + 1) * 128] += (
                        Mw.astype(np.float32).T @ Sd.astype(np.float32)
                    )
                HT[:, gv * GW : (gv + 1) * GW] = np.maximum(
                    acc + b[:, None], 0.0
                )
            Hn.append(HT)
        H = Hn
    out = np.zeros(cfg.n, np.float32)
    for c in range(NC):
        o = H[c].T @ lin_w[:, 0] + lin_b[0]
        out[c * cfg.shard : (c + 1) * cfg.shard] = o[: cfg.shard]
    return out


def build_program(plan: Plan, cfg: Cfg):
    """Build the SPMD Bass program (same NEFF on all 8 cores)."""
    nc = bacc.Bacc(
        "TRN2", target_bir_lowering=False, debug=False, num_devices=NC,
        num_swdge_queues=4, dynamic_dma_scratch_size=16384,
    )
    spad, nwin, tot = cfg.spad, cfg.nwin, plan.tot
    hr = cfg.hrows
    npair = cfg.npair
    MD = cfg.msg_dtype
    nA = cfg.hrowsA // WIN  # windows in table A

    xT = nc.dram_tensor("xT", [D, spad], F16, kind="ExternalInput")
    Wd = [
        nc.dram_tensor(f"W{l}", [D, D], F32, kind="ExternalInput") for l in range(3)
    ]
    bd = [
        nc.dram_tensor(f"b{l}", [D, 1], F32, kind="ExternalInput") for l in range(3)
    ]
    linw_d = nc.dram_tensor("lin_w", [D, 1], F32, kind="ExternalInput")
    linb_d = nc.dram_tensor("lin_b", [D, 1], F32, kind="ExternalInput")
    ident_d = nc.dram_tensor("ident", [D, D], F32, kind="ExternalInput")
    gidx_d = nc.dram_tensor("gidx", [D, tot * 8], I16, kind="ExternalInput")
    sflat_d = nc.dram_tensor("sflat", [D, tot * GW], MD, kind="ExternalInput")
    diag_d = nc.dram_tensor("diag", [D, nwin * WIN], MD, kind="ExternalInput")
    out_d = nc.dram_tensor("out", [nwin, WIN], F32, kind="ExternalOutput")

    with tile.TileContext(nc) as tc, ExitStack() as stk:
        consts = stk.enter_context(tc.tile_pool(name="consts", bufs=1))
        hpool = stk.enter_context(tc.tile_pool(name="hpool", bufs=2))
        ppool = stk.enter_context(tc.tile_pool(name="ppool", bufs=2))
        mpool = stk.enter_context(tc.tile_pool(name="mpool", bufs=5))
        spool = stk.enter_context(tc.tile_pool(name="spool", bufs=5))
        pstage = stk.enter_context(tc.tile_pool(name="pstage", bufs=2))
        psum_agg = stk.enter_context(
            tc.tile_pool(name="psum_agg", bufs=4, space="PSUM")
        )
        psum_p = stk.enter_context(tc.tile_pool(name="psum_p", bufs=2, space="PSUM"))
        dram = stk.enter_context(tc.tile_pool(name="dram", bufs=2, space="DRAM"))

        def load_const(name, dr, shape, dtype):
            t = consts.tile(shape, dtype, name=name)
            nc.sync.dma_start(t[:], dr[tuple(slice(0, s) for s in shape)])
            return t

        ident_sb = load_const("ident_sb", ident_d, [D, D], F32)
        W_sb = []
        for l in range(3):
            wf = load_const(f"W{l}_sbf", Wd[l], [D, D], F32)
            wh = consts.tile([D, D], MD, name=f"W{l}_sb")
            nc.vector.tensor_copy(wh[:], wf[:])
            W_sb.append(wh)
        b_sb = [load_const(f"b{l}_sb", bd[l], [D, 1], F32) for l in range(3)]
        linwf = load_const("linw_sbf", linw_d, [D, 1], F32)
        linw_sb = consts.tile([D, 1], MD, name="linw_sb")
        nc.vector.tensor_copy(linw_sb[:], linwf[:])
        linb_sb = load_const("linb_sb", linb_d, [D, 1], F32)
        gidx_sb = load_const("gidx_sb", gidx_d, [D, tot * 8], I16)
        diag_sb = load_const("diag_sb", diag_d, [D, nwin * WIN], MD)

        def new_ptables(l):
            agin = [
                dram.tile([hr[i], D], MD, tag=f"agin{ab}", name=f"agin{ab}{l}")
                for i, ab in enumerate("AB")
            ]
            pfull = [
                dram.tile(
                    [NC * hr[i], D], MD, tag=f"pfull{ab}", name=f"pfull{ab}{l}",
                    addr_space="Shared",
                )
                for i, ab in enumerate("AB")
            ]
            plocal = ppool.tile([D, nwin, D], MD, tag="plocal", name=f"plocal{l}")
            return agin, pfull, plocal

        def emit_pmm(HTsrc, l, w, agin, plocal):
            """p = H[:, w] @ W_l, cast fp16, into plocal + agin half."""
            pp = psum_p.tile([D, D], F32, tag="pp", name=f"pp{l}_{w}")
            nc.tensor.matmul(
                pp[:], HTsrc[:, w * WIN : (w + 1) * WIN], W_sb[l][:],
                start=True, stop=True,
            )
            nc.vector.tensor_copy(plocal[:, w, :], pp[:])
            hh, wl = (0, w) if w < nA else (1, w - nA)
            nc.sync.dma_start(
                agin[hh][wl * WIN : (wl + 1) * WIN, :], plocal[:, w, :]
            )

        def emit_ag(agin, pfull, hh, l):
            nc.gpsimd.collective_compute(
                "AllGather",
                ALU.bypass,
                replica_groups=[list(range(NC))],
                ins=[agin[hh].opt()],
                outs=[pfull[hh].opt()],
            )

        # ---- prologue: load x, p-mms for layer 0, AGs ----
        HT = hpool.tile([D, spad], MD, tag="HT", name="HT_x")
        xsl = spad // 4
        for s in range(4):
            nc.sync.dma_start(
                HT[:, s * xsl : (s + 1) * xsl], xT[:, s * xsl : (s + 1) * xsl]
            )
        agin, pfull, plocal = new_ptables(0)
        for w in range(nwin):
            emit_pmm(HT, 0, w, agin, plocal)
            if w == nA - 1:
                emit_ag(agin, pfull, 0, 0)
        emit_ag(agin, pfull, 1, 0)

        for l in range(3):
            last = l == 2
            if not last:
                agin_n, pfull_n, plocal_n = new_ptables(l + 1)
            HTn = hpool.tile([D, spad], MD, tag="HT", name=f"HT{l + 1}")
            def gather_h(ch, h, ci):
                nt = ch["nt"][h]
                if nt == 0:
                    return None
                m = mpool.tile(
                    [D, nt, WIN], MD, tag=f"mb{h}",
                    name=f"mb{l}_{ch['t0'][h]}_{h}",
                )
                t0 = ch["t0"][h]
                nc.gpsimd.dma_gather(
                    m[:],
                    pfull[h][:, :],
                    gidx_sb[:, t0 * 8 : (t0 + nt) * 8],
                    nt * 128,
                    nt * 128,
                    D,
                    single_packet=False,
                    queue_num=(2 * ci + h) % 4,
                )
                return m

            def process(ch, mb):
                gv = ch["g"]
                sbase = ch["t0"][0]
                scnt = ch["nt"][0] + ch["nt"][1]
                s_sb = spool.tile(
                    [D, scnt * GW], MD, tag="S", name=f"S{l}_{sbase}"
                )
                nc.sync.dma_start(
                    s_sb[:], sflat_d[:, sbase * GW : (sbase + scnt) * GW]
                )
                ap = psum_agg.tile([D, GW], F32, tag="agg", name=f"agg{l}_{gv}")
                for i, (hh, off, gt) in enumerate(ch["tiles"]):
                    nc.tensor.matmul(
                        ap[:],
                        mb[hh][:, off, :],
                        s_sb[:, (gt - sbase) * GW : (gt - sbase + 1) * GW],
                        start=(i == 0),
                        stop=False,
                    )
                # self-loop terms: p_local windows against diag(dinv^2)
                for j in range(2):
                    wv = 2 * gv + j
                    nc.tensor.matmul(
                        ap[:, j * WIN : (j + 1) * WIN],
                        plocal[:, wv, :],
                        diag_sb[:, wv * WIN : (wv + 1) * WIN],
                        start=False,
                        stop=(j == 1),
                    )
                nc.scalar.activation(
                    HTn[:, gv * GW : (gv + 1) * GW],
                    ap[:],
                    AF.Relu,
                    bias=b_sb[l][:, 0:1],
                )
                if not last:
                    for j in range(2):
                        wv = 2 * gv + j
                        emit_pmm(HTn, l + 1, wv, agin_n, plocal_n)
                        if wv == nA - 1:
                            emit_ag(agin_n, pfull_n, 0, l + 1)

            pend = []
            for ci, ch in enumerate(plan.chunks):
                mA = gather_h(ch, 0, ci)
                pend.append((ch, mA, ci))
                if len(pend) > 1:
                    pch, pA, pci = pend.pop(0)
                    pB = gather_h(pch, 1, pci)
                    process(pch, {0: pA, 1: pB})
            while pend:
                pch, pA, pci = pend.pop(0)
                pB = gather_h(pch, 1, pci)
                process(pch, {0: pA, 1: pB})
            if not last:
                emit_ag(agin_n, pfull_n, 1, l + 1)
                agin, pfull, plocal = agin_n, pfull_n, plocal_n
            HT = HTn

        # ---- head: out = H3 @ lin_w + lin_b ----
        stage = pstage.tile([D, nwin], F32, tag="stage")
        for w in range(nwin):
            op = psum_p.tile([D, 1], F32, tag="op", name=f"op{w}", bufs=1)
            nc.tensor.matmul(
                op[:], HT[:, w * WIN : (w + 1) * WIN], linw_sb[:, :], start=True,
                stop=True,
            )
            nc.vector.tensor_scalar(
                stage[:, w : w + 1], op[:], linb_sb[:, 0:1], None, op0=ALU.add
            )
        tp = psum_p.tile([nwin, D], F32, tag="tp", bufs=1)
        nc.tensor.transpose(tp[:], stage[:], ident_sb[:])
        ov = pstage.tile([nwin, D], F32, tag="ov")
        nc.vector.tensor_copy(ov[:], tp[:])
        nc.sync.dma_start(out_d[:, :], ov[:])

    nc.compile()
    return nc


LAST = {}


def make_in_maps(inputs, per_core, cfg: Cfg):
    x = np.ascontiguousarray(np.asarray(inputs["x"], dtype=np.float32))
    Ws = [np.asarray(inputs[f"W{l}"], dtype=np.float32) for l in range(3)]
    bs = [np.asarray(inputs[f"b{l}"], dtype=np.float32) for l in range(3)]
    lin_w = np.asarray(inputs["lin_w"], dtype=np.float32)
    lin_b = np.asarray(inputs["lin_b"], dtype=np.float32)
    spad = cfg.spad
    ident = np.eye(D, dtype=np.float32)
    in_maps = []
    for c in range(NC):
        xs = x[c * cfg.shard : (c + 1) * cfg.shard]
        xT = np.zeros((D, spad), np.float16)
        xT[:, : cfg.shard] = xs.T.astype(np.float16)
        im = {
            "xT": xT,
            "lin_w": lin_w.astype(np.float32).reshape(D, 1),
            "lin_b": np.full((D, 1), float(lin_b.reshape(-1)[0]), np.float32),
            "ident": ident,
            "gidx": per_core[c]["gidx"],
            "sflat": per_core[c]["sflat"],
            "diag": per_core[c]["diag"],
        }
        for l in range(3):
            im[f"W{l}"] = Ws[l]
            im[f"b{l}"] = bs[l].reshape(D, 1)
        in_maps.append(im)
    return in_maps


def kernel(**inputs):
    cfg = Cfg()
    edge_index = np.asarray(inputs["edge_index"], dtype=np.int32)
    plan, per_core, _ = preprocess(edge_index, cfg)
    nc = build_program(plan, cfg)
    in_maps = make_in_maps(inputs, per_core, cfg)

    res = run_bass_kernel_spmd(nc, in_maps, core_ids=list(range(NC)))
    LAST["res"] = res
    out = np.zeros(cfg.n, np.float32)
    for c in range(NC):
        out[c * cfg.shard : (c + 1) * cfg.shard] = res.results[c]["out"].reshape(-1)[
            : cfg.shard
        ]
    return out


# revision 14
# speedup vs baseline: 1.0825x; 1.0825x over previous
"""Distributed GCN (3x GCNConv + linear head) on 8 TRN2 NeuronCores.

Strategy (graph/data parallel, per sharding hint):
  - Nodes block-sharded across 8 cores (5000 real rows each, padded to 5120).
  - Weights replicated; per-layer: p = H @ W computed locally per 128-node
    window (node-major pc tiles kept in SBUF as `plocal`), cast to fp16 and
    DMA'd into two staging buffers agin_A (local rows 0..2559) / agin_B
    (2560..5119). Two AllGathers (A fires as soon as windows 0..19 are done,
    overlapping the rest of the previous layer's aggregation) build two
    shared tables pfull_A/pfull_B [8*2560, 128] fp16 in DRAM.
  - Edges assigned to the core owning dst; self-loop terms are NOT edges:
    they are added per window with one extra matmul against a host-built
    diagonal S (diag(dinv^2)) using the SBUF-resident plocal tiles.
  - Remaining edges are grouped by (dst PAIR-window of 256 nodes, A/B table)
    and padded to 128-edge tiles. Per-edge norm dinv[src]*dinv[dst] is folded
    into host-built one-hot scatter tiles S [slot, dst-in-pair] fp16 (256
    wide, so each gathered tile needs a single wide matmul).
  - Gather: batched nc.gpsimd.dma_gather (int16 idxs < 20480) across 4 SWDGE
    queues (round-robin; each queue's descriptor generation runs on its own
    Q7 core pair, so up to 4 generations run concurrently) pulls message rows
    M [128 slots, 128 feat] fp16 from pfull_{A,B}; aggregation is
    PSUM[f, 256] += M^T @ S on the PE, flushed with Relu+bias on ACT into the
    next layer's H^T. The next layer's p-matmul for window w is emitted right
    after window w's flush.
  - Head: out = H3 @ lin_w + lin_b via PE + transpose, one [5120] f32 per
    core, host concatenates and trims padding.

Self-contained: hardcodes the problem shapes; all host-side prep derives
from the runtime edge_index only (index bookkeeping + degree).
"""

import os
from contextlib import ExitStack
from dataclasses import dataclass, field

import numpy as np

import concourse.bacc as bacc
import concourse.bass as bass
import concourse.mybir as mybir
import concourse.tile as tile
from concourse.bass_utils import run_bass_kernel_spmd

F32 = mybir.dt.float32
F16 = mybir.dt.float16
I16 = mybir.dt.int16
AF = mybir.ActivationFunctionType
ALU = mybir.AluOpType

D = 128  # feature dim (in = hid = 128)
WIN = 128  # dst nodes per flush window
GW = 256  # dst nodes per aggregation (pair) group / S width
NC = 8  # cores


@dataclass
class Cfg:
    n: int = 40000
    e: int = 640000
    shard: int = 5000  # real nodes per core
    msg_dtype: object = F16

    @property
    def spad(self):  # padded shard
        return ((self.shard + GW - 1) // GW) * GW

    @property
    def nwin(self):
        return self.spad // WIN

    @property
    def npair(self):
        return self.spad // GW

    @property
    def hrows(self):  # local rows per A/B table half
        return self.spad // 2

    @property
    def half(self):  # rows per gather table (pfull_A or pfull_B)
        return NC * self.spad // 2


@dataclass
class Plan:
    """Per-call schedule shared by all cores (static SPMD program)."""

    caps: np.ndarray  # [npair, 2] tiles per (pair, half), max over cores
    tot: int  # total tiles per layer
    chunks: list = field(default_factory=list)
    # chunks: one per pair group g:
    #  {"g": g, "nt": {h: ntiles}, "t0": {h: first-global-tile},
    #   "tiles": [(h, off_in_half_buf, global_tile), ...]}


def build_plan(caps: np.ndarray, cfg: Cfg) -> Plan:
    plan = Plan(caps=caps, tot=int(caps.sum()))
    t = 0
    for g in range(cfg.npair):
        ch = {"g": g, "nt": {}, "t0": {}, "tiles": []}
        for h in (0, 1):
            ch["t0"][h] = t
            off = 0
            for _ in range(int(caps[g, h])):
                ch["tiles"].append((h, off, t))
                off += 1
                t += 1
            ch["nt"][h] = off
        plan.chunks.append(ch)
    assert t == plan.tot
    return plan


def preprocess(edge_index: np.ndarray, cfg: Cfg):
    """Host-side index prep. Returns (plan, per_core dict arrays, dinv)."""
    n, shard, spad, hrows = cfg.n, cfg.shard, cfg.spad, cfg.hrows
    npair, nwin = cfg.npair, cfg.nwin
    src = edge_index[0].astype(np.int64)
    dst = edge_index[1].astype(np.int64)
    deg = 1.0 + np.bincount(dst, minlength=n).astype(np.float64)
    dinv = (1.0 / np.sqrt(deg)).astype(np.float32)

    allnorm = (dinv[src] * dinv[dst]).astype(np.float32)

    core = dst // shard
    dloc = dst % shard
    g = dloc // GW
    dwin = (dloc % GW).astype(np.float32)
    sc = src // shard
    sloc = src % shard
    h = sloc // hrows  # 0 = table A (local rows < hrows), 1 = table B
    idx = (sc * hrows + (sloc % hrows)).astype(np.int16)
    assert NC * hrows <= 32768

    key = (core * npair + g) * 2 + h
    cnt = np.bincount(key, minlength=NC * npair * 2).reshape(NC, npair, 2)
    caps = np.ceil(cnt.max(axis=0) / 128.0).astype(np.int64)  # [npair, 2]
    caps = np.maximum(caps, 1)
    plan = build_plan(caps, cfg)
    tot = plan.tot

    order = np.lexsort((h, g, core))
    osrcidx = idx[order]
    odwin = dwin[order]
    onorm = allnorm[order]
    okey = key[order]
    starts = np.zeros(NC * npair * 2 + 1, dtype=np.int64)
    np.cumsum(np.bincount(okey, minlength=NC * npair * 2), out=starts[1:])

    # first tile of each (pair, half) group in the tile stream
    gslot = np.zeros((npair, 2), dtype=np.int64)
    for ch in plan.chunks:
        firsts = {}
        for hh, _off, gt in ch["tiles"]:
            if hh not in firsts:
                firsts[hh] = gt
        for hh, gt in firsts.items():
            gslot[ch["g"], hh] = gt

    per_core = []
    for c in range(NC):
        gi = np.zeros(tot * 128, dtype=np.int16)
        dl = np.zeros(tot * 128, dtype=np.float32)
        nv = np.zeros(tot * 128, dtype=np.float32)
        for gv in range(npair):
            for hh in (0, 1):
                k = (c * npair + gv) * 2 + hh
                s, e_ = starts[k], starts[k + 1]
                m = e_ - s
                if m == 0:
                    continue
                base = gslot[gv, hh] * 128
                assert m <= caps[gv, hh] * 128
                gi[base : base + m] = osrcidx[s:e_]
                dl[base : base + m] = odwin[s:e_]
                nv[base : base + m] = onorm[s:e_]
        # gather idx layout: idx i -> [i%16 (+16k replicas), i//16]
        gi16 = gi.reshape(tot * 8, 16).T  # [16, tot*8]
        gi128 = np.tile(gi16, (8, 1)).copy()  # [128, tot*8]
        # host-prebuilt scatter one-hots S [tot*128, GW] -> partition-major
        sf = np.zeros((tot * 128, GW), np.float16)
        sf[np.arange(tot * 128), dl.astype(np.int64)] = nv.astype(np.float16)
        sflat = np.ascontiguousarray(
            sf.reshape(tot, 128, GW).transpose(1, 0, 2).reshape(128, tot * GW)
        )
        # self-loop diagonal S per window: diag(dinv^2) over local rows
        dg = np.zeros((128, nwin * 128), np.float16)
        for wv in range(nwin):
            rows = np.arange(wv * 128, (wv + 1) * 128) + c * shard
            val = np.where(
                np.arange(wv * 128, (wv + 1) * 128) < shard,
                (dinv[np.minimum(rows, n - 1)] ** 2),
                0.0,
            ).astype(np.float16)
            dg[np.arange(128), wv * 128 + np.arange(128)] = val
        per_core.append({"gidx": gi128, "sflat": sflat, "diag": dg})
    return plan, per_core, dinv


def emulate(x, edge_index, Ws, bs, lin_w, lin_b, cfg: Cfg, fp16=True):
    """Numpy emulation of the exact device dataflow (for validation)."""
    plan, per_core, dinv = preprocess(edge_index, cfg)
    spad, nwin, hrows = cfg.spad, cfg.nwin, cfg.hrows
    md = np.float16 if fp16 else np.float32
    H = []
    for c in range(NC):
        xs = x[c * cfg.shard : (c + 1) * cfg.shard]
        H.append(
            np.concatenate([xs, np.zeros((spad - cfg.shard, D), np.float32)]).T.copy()
        )
    for l in range(3):
        W, b = Ws[l], bs[l]
        pf = [np.zeros((NC * hrows, D), md) for _ in range(2)]
        plocal = []
        for c in range(NC):
            p = (H[c].T.astype(np.float32) @ W).astype(md)
            pf[0][c * hrows : (c + 1) * hrows] = p[:hrows]
            pf[1][c * hrows : (c + 1) * hrows] = p[hrows:]
            plocal.append(p)
        Hn = []
        for c in range(NC):
            pc = per_core[c]
            HT = np.zeros((D, spad), np.float32)
            for ch in plan.chunks:
                gv = ch["g"]
                acc = np.zeros((D, GW), np.float32)
                for hh, _off, gt in ch["tiles"]:
                    ii = pc["gidx"][:16, gt * 8 : gt * 8 + 8].T.reshape(-1)
                    M = pf[hh][ii.astype(np.int64)]
                    S = pc["sflat"][:, gt * GW : (gt + 1) * GW]
                    acc += M.astype(np.float32).T @ S.astype(np.float32)
                for j in range(2):
                    wv = 2 * gv + j
                    Mw = plocal[c][wv * 128 : (wv + 1) * 128]
                    Sd = pc["diag"][:, wv * 128 : (wv + 1) * 128]
                    acc[:, j * 128 : (j + 1) * 128] += (
                        Mw.astype(np.float32).T @ Sd.astype(np.float32)
                    )
                HT[:, gv * GW : (gv + 1) * GW] = np.maximum(
                    acc + b[:, None], 0.0
                )
            Hn.append(HT)
        H = Hn
    out = np.zeros(cfg.n, np.float32)
    for c in range(NC):
        o = H[c].T @ lin_w[:, 0] + lin_b[0]
        out[c * cfg.shard : (c + 1) * cfg.shard] = o[: cfg.shard]
    return out


def build_program(plan: Plan, cfg: Cfg):
    """Build the SPMD Bass program (same NEFF on all 8 cores)."""
    nc = bacc.Bacc(
        "TRN2", target_bir_lowering=False, debug=False, num_devices=NC,
        num_swdge_queues=4, dynamic_dma_scratch_size=16384,
    )
    spad, nwin, hrows, half, tot = cfg.spad, cfg.nwin, cfg.hrows, cfg.half, plan.tot
    npair = cfg.npair
    MD = cfg.msg_dtype
    nA = nwin // 2  # windows in table A

    xT = nc.dram_tensor("xT", [D, spad], F16, kind="ExternalInput")
    Wd = [
        nc.dram_tensor(f"W{l}", [D, D], F32, kind="ExternalInput") for l in range(3)
    ]
    bd = [
        nc.dram_tensor(f"b{l}", [D, 1], F32, kind="ExternalInput") for l in range(3)
    ]
    linw_d = nc.dram_tensor("lin_w", [D, 1], F32, kind="ExternalInput")
    linb_d = nc.dram_tensor("lin_b", [D, 1], F32, kind="ExternalInput")
    ident_d = nc.dram_tensor("ident", [D, D], F32, kind="ExternalInput")
    gidx_d = nc.dram_tensor("gidx", [D, tot * 8], I16, kind="ExternalInput")
    sflat_d = nc.dram_tensor("sflat", [D, tot * GW], MD, kind="ExternalInput")
    diag_d = nc.dram_tensor("diag", [D, nwin * WIN], MD, kind="ExternalInput")
    out_d = nc.dram_tensor("out", [nwin, WIN], F32, kind="ExternalOutput")

    with tile.TileContext(nc) as tc, ExitStack() as stk:
        consts = stk.enter_context(tc.tile_pool(name="consts", bufs=1))
        hpool = stk.enter_context(tc.tile_pool(name="hpool", bufs=2))
        ppool = stk.enter_context(tc.tile_pool(name="ppool", bufs=2))
        mpool = stk.enter_context(tc.tile_pool(name="mpool", bufs=5))
        spool = stk.enter_context(tc.tile_pool(name="spool", bufs=5))
        pstage = stk.enter_context(tc.tile_pool(name="pstage", bufs=2))
        psum_agg = stk.enter_context(
            tc.tile_pool(name="psum_agg", bufs=4, space="PSUM")
        )
        psum_p = stk.enter_context(tc.tile_pool(name="psum_p", bufs=2, space="PSUM"))
        dram = stk.enter_context(tc.tile_pool(name="dram", bufs=2, space="DRAM"))

        def load_const(name, dr, shape, dtype):
            t = consts.tile(shape, dtype, name=name)
            nc.sync.dma_start(t[:], dr[tuple(slice(0, s) for s in shape)])
            return t

        ident_sb = load_const("ident_sb", ident_d, [D, D], F32)
        W_sb = []
        for l in range(3):
            wf = load_const(f"W{l}_sbf", Wd[l], [D, D], F32)
            wh = consts.tile([D, D], MD, name=f"W{l}_sb")
            nc.vector.tensor_copy(wh[:], wf[:])
            W_sb.append(wh)
        b_sb = [load_const(f"b{l}_sb", bd[l], [D, 1], F32) for l in range(3)]
        linwf = load_const("linw_sbf", linw_d, [D, 1], F32)
        linw_sb = consts.tile([D, 1], MD, name="linw_sb")
        nc.vector.tensor_copy(linw_sb[:], linwf[:])
        linb_sb = load_const("linb_sb", linb_d, [D, 1], F32)
        gidx_sb = load_const("gidx_sb", gidx_d, [D, tot * 8], I16)
        diag_sb = load_const("diag_sb", diag_d, [D, nwin * WIN], MD)

        def new_ptables(l):
            agin = [
                dram.tile([hrows, D], MD, tag=f"agin{ab}", name=f"agin{ab}{l}")
                for ab in "AB"
            ]
            pfull = [
                dram.tile(
                    [half, D], MD, tag=f"pfull{ab}", name=f"pfull{ab}{l}",
                    addr_space="Shared",
                )
                for ab in "AB"
            ]
            plocal = ppool.tile([D, nwin, D], MD, tag="plocal", name=f"plocal{l}")
            return agin, pfull, plocal

        def emit_pmm(HTsrc, l, w, agin, plocal):
            """p = H[:, w] @ W_l, cast fp16, into plocal + agin half."""
            pp = psum_p.tile([D, D], F32, tag="pp", name=f"pp{l}_{w}")
            nc.tensor.matmul(
                pp[:], HTsrc[:, w * WIN : (w + 1) * WIN], W_sb[l][:],
                start=True, stop=True,
            )
            nc.vector.tensor_copy(plocal[:, w, :], pp[:])
            hh, wl = (0, w) if w < nA else (1, w - nA)
            nc.sync.dma_start(
                agin[hh][wl * WIN : (wl + 1) * WIN, :], plocal[:, w, :]
            )

        def emit_ag(agin, pfull, hh, l):
            nc.gpsimd.collective_compute(
                "AllGather",
                ALU.bypass,
                replica_groups=[list(range(NC))],
                ins=[agin[hh].opt()],
                outs=[pfull[hh].opt()],
            )

        # ---- prologue: load x, p-mms for layer 0, AGs ----
        HT = hpool.tile([D, spad], MD, tag="HT", name="HT_x")
        xsl = spad // 4
        for s in range(4):
            nc.sync.dma_start(
                HT[:, s * xsl : (s + 1) * xsl], xT[:, s * xsl : (s + 1) * xsl]
            )
        agin, pfull, plocal = new_ptables(0)
        for w in range(nwin):
            emit_pmm(HT, 0, w, agin, plocal)
            if w == nA - 1:
                emit_ag(agin, pfull, 0, 0)
        emit_ag(agin, pfull, 1, 0)

        for l in range(3):
            last = l == 2
            if not last:
                agin_n, pfull_n, plocal_n = new_ptables(l + 1)
            HTn = hpool.tile([D, spad], MD, tag="HT", name=f"HT{l + 1}")
            def gather_h(ch, h, ci):
                nt = ch["nt"][h]
                if nt == 0:
                    return None
                m = mpool.tile(
                    [D, nt, WIN], MD, tag=f"mb{h}",
                    name=f"mb{l}_{ch['t0'][h]}_{h}",
                )
                t0 = ch["t0"][h]
                nc.gpsimd.dma_gather(
                    m[:],
                    pfull[h][:, :],
                    gidx_sb[:, t0 * 8 : (t0 + nt) * 8],
                    nt * 128,
                    nt * 128,
                    D,
                    single_packet=False,
                    queue_num=(2 * ci + h) % 4,
                )
                return m

            def process(ch, mb):
                gv = ch["g"]
                sbase = ch["t0"][0]
                scnt = ch["nt"][0] + ch["nt"][1]
                s_sb = spool.tile(
                    [D, scnt * GW], MD, tag="S", name=f"S{l}_{sbase}"
                )
                nc.sync.dma_start(
                    s_sb[:], sflat_d[:, sbase * GW : (sbase + scnt) * GW]
                )
                ap = psum_agg.tile([D, GW], F32, tag="agg", name=f"agg{l}_{gv}")
                for i, (hh, off, gt) in enumerate(ch["tiles"]):
                    nc.tensor.matmul(
                        ap[:],
                        mb[hh][:, off, :],
                        s_sb[:, (gt - sbase) * GW : (gt - sbase + 1) * GW],
                        start=(i == 0),
                        stop=False,
                    )
                # self-loop terms: p_local windows against diag(dinv^2)
                for j in range(2):
                    wv = 2 * gv + j
                    nc.tensor.matmul(
                        ap[:, j * WIN : (j + 1) * WIN],
                        plocal[:, wv, :],
                        diag_sb[:, wv * WIN : (wv + 1) * WIN],
                        start=False,
                        stop=(j == 1),
                    )
                nc.scalar.activation(
                    HTn[:, gv * GW : (gv + 1) * GW],
                    ap[:],
                    AF.Relu,
                    bias=b_sb[l][:, 0:1],
                )
                if not last:
                    for j in range(2):
                        wv = 2 * gv + j
                        emit_pmm(HTn, l + 1, wv, agin_n, plocal_n)
                        if wv == nA - 1:
                            emit_ag(agin_n, pfull_n, 0, l + 1)

            pend = []
            for ci, ch in enumerate(plan.chunks):
                mA = gather_h(ch, 0, ci)
                pend.append((ch, mA, ci))
                if len(pend) > 1:
                    pch, pA, pci = pend.pop(0)
                    pB = gather_h(pch, 1, pci)
                    process(pch, {0: pA, 1: pB})
            while pend:
                pch, pA, pci = pend.pop(0)
                pB = gather_h(pch, 1, pci)
                process(pch, {0: pA, 1: pB})
            if not last:
                emit_ag(agin_n, pfull_n, 1, l + 1)
                agin, pfull, plocal = agin_n, pfull_n, plocal_n
            HT = HTn

        # ---- head: out = H3 @ lin_w + lin_b ----
        stage = pstage.tile([D, nwin], F32, tag="stage")
        for w in range(nwin):
            op = psum_p.tile([D, 1], F32, tag="op", name=f"op{w}", bufs=1)
            nc.tensor.matmul(
                op[:], HT[:, w * WIN : (w + 1) * WIN], linw_sb[:, :], start=True,
                stop=True,
            )
            nc.vector.tensor_scalar(
                stage[:, w : w + 1], op[:], linb_sb[:, 0:1], None, op0=ALU.add
            )
        tp = psum_p.tile([nwin, D], F32, tag="tp", bufs=1)
        nc.tensor.transpose(tp[:], stage[:], ident_sb[:])
        ov = pstage.tile([nwin, D], F32, tag="ov")
        nc.vector.tensor_copy(ov[:], tp[:])
        nc.sync.dma_start(out_d[:, :], ov[:])

    nc.compile()
    return nc


LAST = {}


def make_in_maps(inputs, per_core, cfg: Cfg):
    x = np.ascontiguousarray(np.asarray(inputs["x"], dtype=np.float32))
    Ws = [np.asarray(inputs[f"W{l}"], dtype=np.float32) for l in range(3)]
    bs = [np.asarray(inputs[f"b{l}"], dtype=np.float32) for l in range(3)]
    lin_w = np.asarray(inputs["lin_w"], dtype=np.float32)
    lin_b = np.asarray(inputs["lin_b"], dtype=np.float32)
    spad = cfg.spad
    ident = np.eye(D, dtype=np.float32)
    in_maps = []
    for c in range(NC):
        xs = x[c * cfg.shard : (c + 1) * cfg.shard]
        xT = np.zeros((D, spad), np.float16)
        xT[:, : cfg.shard] = xs.T.astype(np.float16)
        im = {
            "xT": xT,
            "lin_w": lin_w.astype(np.float32).reshape(D, 1),
            "lin_b": np.full((D, 1), float(lin_b.reshape(-1)[0]), np.float32),
            "ident": ident,
            "gidx": per_core[c]["gidx"],
            "sflat": per_core[c]["sflat"],
            "diag": per_core[c]["diag"],
        }
        for l in range(3):
            im[f"W{l}"] = Ws[l]
            im[f"b{l}"] = bs[l].reshape(D, 1)
        in_maps.append(im)
    return in_maps


def kernel(**inputs):
    cfg = Cfg()
    edge_index = np.asarray(inputs["edge_index"], dtype=np.int32)
    plan, per_core, _ = preprocess(edge_index, cfg)
    nc = build_program(plan, cfg)
    in_maps = make_in_maps(inputs, per_core, cfg)

    res = run_bass_kernel_spmd(nc, in_maps, core_ids=list(range(NC)))
    LAST["res"] = res
    out = np.zeros(cfg.n, np.float32)
    for c in range(NC):
        out[c * cfg.shard : (c + 1) * cfg.shard] = res.results[c]["out"].reshape(-1)[
            : cfg.shard
        ]
    return out


# revision 16
# speedup vs baseline: 1.0930x; 1.0098x over previous
"""Distributed GCN (3x GCNConv + linear head) on 8 TRN2 NeuronCores.

Strategy (graph/data parallel, per sharding hint):
  - Nodes block-sharded across 8 cores (5000 real rows each, padded to 5120).
  - Weights replicated; per-layer: p = H @ W computed locally per 128-node
    window (node-major pc tiles kept in SBUF as `plocal`), cast to fp16 and
    DMA'd into two staging buffers agin_A (local rows 0..2559) / agin_B
    (2560..5119). Two AllGathers (A fires as soon as windows 0..19 are done,
    overlapping the rest of the previous layer's aggregation) build two
    shared tables pfull_A/pfull_B [8*2560, 128] fp16 in DRAM.
  - Edges assigned to the core owning dst; self-loop terms are NOT edges:
    they are added per window with one extra matmul against a host-built
    diagonal S (diag(dinv^2)) using the SBUF-resident plocal tiles.
  - Remaining edges are grouped by (dst PAIR-window of 256 nodes, A/B table)
    and padded to 128-edge tiles. Per-edge norm dinv[src]*dinv[dst] is folded
    into host-built one-hot scatter tiles S [slot, dst-in-pair] fp16 (256
    wide, so each gathered tile needs a single wide matmul).
  - Gather: batched nc.gpsimd.dma_gather (int16 idxs < 20480) across 4 SWDGE
    queues (round-robin; each queue's descriptor generation runs on its own
    Q7 core pair, so up to 4 generations run concurrently) pulls message rows
    M [128 slots, 128 feat] fp16 from pfull_{A,B}; aggregation is
    PSUM[f, 256] += M^T @ S on the PE, flushed with Relu+bias on ACT into the
    next layer's H^T. The next layer's p-matmul for window w is emitted right
    after window w's flush.
  - Head: out = H3 @ lin_w + lin_b via PE + transpose, one [5120] f32 per
    core, host concatenates and trims padding.

Self-contained: hardcodes the problem shapes; all host-side prep derives
from the runtime edge_index only (index bookkeeping + degree).
"""

import os
from contextlib import ExitStack
from dataclasses import dataclass, field

import numpy as np

import concourse.bacc as bacc
import concourse.bass as bass
import concourse.mybir as mybir
import concourse.tile as tile
from concourse.bass_utils import run_bass_kernel_spmd

F32 = mybir.dt.float32
F16 = mybir.dt.float16
I16 = mybir.dt.int16
AF = mybir.ActivationFunctionType
ALU = mybir.AluOpType

D = 128  # feature dim (in = hid = 128)
WIN = 128  # dst nodes per flush window
GW = 256  # dst nodes per aggregation (pair) group / S width
NC = 8  # cores


@dataclass
class Cfg:
    n: int = 40000
    e: int = 640000
    shard: int = 5000  # real nodes per core
    msg_dtype: object = F16

    @property
    def spad(self):  # padded shard
        return ((self.shard + GW - 1) // GW) * GW

    @property
    def nwin(self):
        return self.spad // WIN

    @property
    def npair(self):
        return self.spad // GW

    @property
    def hrows(self):  # local rows per A/B table half
        return self.spad // 2

    @property
    def half(self):  # rows per gather table (pfull_A or pfull_B)
        return NC * self.spad // 2


@dataclass
class Plan:
    """Per-call schedule shared by all cores (static SPMD program)."""

    caps: np.ndarray  # [npair, 2] tiles per (pair, half), max over cores
    tot: int  # total tiles per layer
    chunks: list = field(default_factory=list)
    # chunks: one per pair group g:
    #  {"g": g, "nt": {h: ntiles}, "t0": {h: first-global-tile},
    #   "tiles": [(h, off_in_half_buf, global_tile), ...]}


def build_plan(caps: np.ndarray, cfg: Cfg) -> Plan:
    plan = Plan(caps=caps, tot=int(caps.sum()))
    t = 0
    for g in range(cfg.npair):
        ch = {"g": g, "nt": {}, "t0": {}, "tiles": []}
        for h in (0, 1):
            ch["t0"][h] = t
            off = 0
            for _ in range(int(caps[g, h])):
                ch["tiles"].append((h, off, t))
                off += 1
                t += 1
            ch["nt"][h] = off
        plan.chunks.append(ch)
    assert t == plan.tot
    return plan


def preprocess(edge_index: np.ndarray, cfg: Cfg):
    """Host-side index prep. Returns (plan, per_core dict arrays, dinv)."""
    n, shard, spad, hrows = cfg.n, cfg.shard, cfg.spad, cfg.hrows
    npair, nwin = cfg.npair, cfg.nwin
    src = edge_index[0].astype(np.int64)
    dst = edge_index[1].astype(np.int64)
    deg = 1.0 + np.bincount(dst, minlength=n).astype(np.float64)
    dinv = (1.0 / np.sqrt(deg)).astype(np.float32)

    allnorm = (dinv[src] * dinv[dst]).astype(np.float32)

    core = dst // shard
    dloc = dst % shard
    g = dloc // GW
    dwin = (dloc % GW).astype(np.float32)
    sc = src // shard
    sloc = src % shard
    h = sloc // hrows  # 0 = table A (local rows < hrows), 1 = table B
    idx = (sc * hrows + (sloc % hrows)).astype(np.int16)
    assert NC * hrows <= 32768

    key = (core * npair + g) * 2 + h
    cnt = np.bincount(key, minlength=NC * npair * 2).reshape(NC, npair, 2)
    caps = np.ceil(cnt.max(axis=0) / 128.0).astype(np.int64)  # [npair, 2]
    caps = np.maximum(caps, 1)
    plan = build_plan(caps, cfg)
    tot = plan.tot

    order = np.lexsort((h, g, core))
    osrcidx = idx[order]
    odwin = dwin[order]
    onorm = allnorm[order]
    okey = key[order]
    starts = np.zeros(NC * npair * 2 + 1, dtype=np.int64)
    np.cumsum(np.bincount(okey, minlength=NC * npair * 2), out=starts[1:])

    # first tile of each (pair, half) group in the tile stream
    gslot = np.zeros((npair, 2), dtype=np.int64)
    for ch in plan.chunks:
        firsts = {}
        for hh, _off, gt in ch["tiles"]:
            if hh not in firsts:
                firsts[hh] = gt
        for hh, gt in firsts.items():
            gslot[ch["g"], hh] = gt

    per_core = []
    for c in range(NC):
        gi = np.zeros(tot * 128, dtype=np.int16)
        dl = np.zeros(tot * 128, dtype=np.float32)
        nv = np.zeros(tot * 128, dtype=np.float32)
        for gv in range(npair):
            for hh in (0, 1):
                k = (c * npair + gv) * 2 + hh
                s, e_ = starts[k], starts[k + 1]
                m = e_ - s
                if m == 0:
                    continue
                base = gslot[gv, hh] * 128
                assert m <= caps[gv, hh] * 128
                gi[base : base + m] = osrcidx[s:e_]
                dl[base : base + m] = odwin[s:e_]
                nv[base : base + m] = onorm[s:e_]
        # gather idx layout: idx i -> [i%16 (+16k replicas), i//16]
        gi16 = gi.reshape(tot * 8, 16).T  # [16, tot*8]
        gi128 = np.tile(gi16, (8, 1)).copy()  # [128, tot*8]
        # host-prebuilt scatter one-hots S [tot*128, GW] -> partition-major
        sf = np.zeros((tot * 128, GW), np.float16)
        sf[np.arange(tot * 128), dl.astype(np.int64)] = nv.astype(np.float16)
        sflat = np.ascontiguousarray(
            sf.reshape(tot, 128, GW).transpose(1, 0, 2).reshape(128, tot * GW)
        )
        # self-loop diagonal S per window: diag(dinv^2) over local rows
        dg = np.zeros((128, nwin * 128), np.float16)
        for wv in range(nwin):
            rows = np.arange(wv * 128, (wv + 1) * 128) + c * shard
            val = np.where(
                np.arange(wv * 128, (wv + 1) * 128) < shard,
                (dinv[np.minimum(rows, n - 1)] ** 2),
                0.0,
            ).astype(np.float16)
            dg[np.arange(128), wv * 128 + np.arange(128)] = val
        per_core.append({"gidx": gi128, "sflat": sflat, "diag": dg})
    return plan, per_core, dinv


def emulate(x, edge_index, Ws, bs, lin_w, lin_b, cfg: Cfg, fp16=True):
    """Numpy emulation of the exact device dataflow (for validation)."""
    plan, per_core, dinv = preprocess(edge_index, cfg)
    spad, nwin, hrows = cfg.spad, cfg.nwin, cfg.hrows
    md = np.float16 if fp16 else np.float32
    H = []
    for c in range(NC):
        xs = x[c * cfg.shard : (c + 1) * cfg.shard]
        H.append(
            np.concatenate([xs, np.zeros((spad - cfg.shard, D), np.float32)]).T.copy()
        )
    for l in range(3):
        W, b = Ws[l], bs[l]
        pf = [np.zeros((NC * hrows, D), md) for _ in range(2)]
        plocal = []
        for c in range(NC):
            p = (H[c].T.astype(np.float32) @ W).astype(md)
            pf[0][c * hrows : (c + 1) * hrows] = p[:hrows]
            pf[1][c * hrows : (c + 1) * hrows] = p[hrows:]
            plocal.append(p)
        Hn = []
        for c in range(NC):
            pc = per_core[c]
            HT = np.zeros((D, spad), np.float32)
            for ch in plan.chunks:
                gv = ch["g"]
                acc = np.zeros((D, GW), np.float32)
                for hh, _off, gt in ch["tiles"]:
                    ii = pc["gidx"][:16, gt * 8 : gt * 8 + 8].T.reshape(-1)
                    M = pf[hh][ii.astype(np.int64)]
                    S = pc["sflat"][:, gt * GW : (gt + 1) * GW]
                    acc += M.astype(np.float32).T @ S.astype(np.float32)
                for j in range(2):
                    wv = 2 * gv + j
                    Mw = plocal[c][wv * 128 : (wv + 1) * 128]
                    Sd = pc["diag"][:, wv * 128 : (wv + 1) * 128]
                    acc[:, j * 128 : (j + 1) * 128] += (
                        Mw.astype(np.float32).T @ Sd.astype(np.float32)
                    )
                HT[:, gv * GW : (gv + 1) * GW] = np.maximum(
                    acc + b[:, None], 0.0
                )
            Hn.append(HT)
        H = Hn
    out = np.zeros(cfg.n, np.float32)
    for c in range(NC):
        o = H[c].T @ lin_w[:, 0] + lin_b[0]
        out[c * cfg.shard : (c + 1) * cfg.shard] = o[: cfg.shard]
    return out


def build_program(plan: Plan, cfg: Cfg):
    """Build the SPMD Bass program (same NEFF on all 8 cores)."""
    nc = bacc.Bacc(
        "TRN2", target_bir_lowering=False, debug=False, num_devices=NC,
        num_swdge_queues=4, dynamic_dma_scratch_size=16384,
    )
    spad, nwin, hrows, half, tot = cfg.spad, cfg.nwin, cfg.hrows, cfg.half, plan.tot
    npair = cfg.npair
    MD = cfg.msg_dtype
    nA = nwin // 2  # windows in table A

    xT = nc.dram_tensor("xT", [D, spad], F16, kind="ExternalInput")
    Wd = [
        nc.dram_tensor(f"W{l}", [D, D], F32, kind="ExternalInput") for l in range(3)
    ]
    bd = [
        nc.dram_tensor(f"b{l}", [D, 1], F32, kind="ExternalInput") for l in range(3)
    ]
    linw_d = nc.dram_tensor("lin_w", [D, 1], F32, kind="ExternalInput")
    linb_d = nc.dram_tensor("lin_b", [D, 1], F32, kind="ExternalInput")
    ident_d = nc.dram_tensor("ident", [D, D], F32, kind="ExternalInput")
    gidx_d = nc.dram_tensor("gidx", [D, tot * 8], I16, kind="ExternalInput")
    sflat_d = nc.dram_tensor("sflat", [D, tot * GW], MD, kind="ExternalInput")
    diag_d = nc.dram_tensor("diag", [D, nwin * WIN], MD, kind="ExternalInput")
    out_d = nc.dram_tensor("out", [nwin, WIN], F32, kind="ExternalOutput")

    with tile.TileContext(nc) as tc, ExitStack() as stk:
        consts = stk.enter_context(tc.tile_pool(name="consts", bufs=1))
        hpool = stk.enter_context(tc.tile_pool(name="hpool", bufs=2))
        ppool = stk.enter_context(tc.tile_pool(name="ppool", bufs=2))
        mpool = stk.enter_context(tc.tile_pool(name="mpool", bufs=5))
        spool = stk.enter_context(tc.tile_pool(name="spool", bufs=5))
        pstage = stk.enter_context(tc.tile_pool(name="pstage", bufs=2))
        psum_agg = stk.enter_context(
            tc.tile_pool(name="psum_agg", bufs=4, space="PSUM")
        )
        psum_p = stk.enter_context(tc.tile_pool(name="psum_p", bufs=2, space="PSUM"))
        dram = stk.enter_context(tc.tile_pool(name="dram", bufs=2, space="DRAM"))

        def load_const(name, dr, shape, dtype):
            t = consts.tile(shape, dtype, name=name)
            nc.sync.dma_start(t[:], dr[tuple(slice(0, s) for s in shape)])
            return t

        ident_sb = load_const("ident_sb", ident_d, [D, D], F32)
        W_sb = []
        for l in range(3):
            wf = load_const(f"W{l}_sbf", Wd[l], [D, D], F32)
            wh = consts.tile([D, D], MD, name=f"W{l}_sb")
            nc.vector.tensor_copy(wh[:], wf[:])
            W_sb.append(wh)
        b_sb = [load_const(f"b{l}_sb", bd[l], [D, 1], F32) for l in range(3)]
        linwf = load_const("linw_sbf", linw_d, [D, 1], F32)
        linw_sb = consts.tile([D, 1], MD, name="linw_sb")
        nc.vector.tensor_copy(linw_sb[:], linwf[:])
        linb_sb = load_const("linb_sb", linb_d, [D, 1], F32)
        gidx_sb = load_const("gidx_sb", gidx_d, [D, tot * 8], I16)
        diag_sb = load_const("diag_sb", diag_d, [D, nwin * WIN], MD)

        def new_ptables(l):
            agin = [
                dram.tile([hrows, D], MD, tag=f"agin{ab}", name=f"agin{ab}{l}")
                for ab in "AB"
            ]
            pfull = [
                dram.tile(
                    [half, D], MD, tag=f"pfull{ab}", name=f"pfull{ab}{l}",
                    addr_space="Shared",
                )
                for ab in "AB"
            ]
            plocal = ppool.tile([D, nwin, D], MD, tag="plocal", name=f"plocal{l}")
            return agin, pfull, plocal

        def emit_pmm(HTsrc, l, w, agin, plocal):
            """p = H[:, w] @ W_l, cast fp16, into plocal + agin half."""
            pp = psum_p.tile([D, D], F32, tag="pp", name=f"pp{l}_{w}")
            nc.tensor.matmul(
                pp[:], HTsrc[:, w * WIN : (w + 1) * WIN], W_sb[l][:],
                start=True, stop=True,
            )
            nc.vector.tensor_copy(plocal[:, w, :], pp[:])
            hh, wl = (0, w) if w < nA else (1, w - nA)
            nc.sync.dma_start(
                agin[hh][wl * WIN : (wl + 1) * WIN, :], plocal[:, w, :]
            )

        def emit_ag(agin, pfull, hh, l):
            nc.gpsimd.collective_compute(
                "AllGather",
                ALU.bypass,
                replica_groups=[list(range(NC))],
                ins=[agin[hh].opt()],
                outs=[pfull[hh].opt()],
            )

        # ---- prologue: load x, p-mms for layer 0, AGs ----
        HT = hpool.tile([D, spad], MD, tag="HT", name="HT_x")
        xsl = spad // 4
        for s in range(4):
            nc.sync.dma_start(
                HT[:, s * xsl : (s + 1) * xsl], xT[:, s * xsl : (s + 1) * xsl]
            )
        agin, pfull, plocal = new_ptables(0)
        for w in range(nwin):
            emit_pmm(HT, 0, w, agin, plocal)
            if w == nA - 1:
                emit_ag(agin, pfull, 0, 0)
        emit_ag(agin, pfull, 1, 0)

        for l in range(3):
            last = l == 2
            if not last:
                agin_n, pfull_n, plocal_n = new_ptables(l + 1)
            HTn = hpool.tile([D, spad], MD, tag="HT", name=f"HT{l + 1}")
            def gather_h(ch, h, ci):
                nt = ch["nt"][h]
                if nt == 0:
                    return None
                m = mpool.tile(
                    [D, nt, WIN], MD, tag=f"mb{h}",
                    name=f"mb{l}_{ch['t0'][h]}_{h}",
                )
                t0 = ch["t0"][h]
                nc.gpsimd.dma_gather(
                    m[:],
                    pfull[h][:, :],
                    gidx_sb[:, t0 * 8 : (t0 + nt) * 8],
                    nt * 128,
                    nt * 128,
                    D,
                    single_packet=False,
                    queue_num=(2 * ci + h) % 4,
                )
                return m

            def process(ch, mb):
                gv = ch["g"]
                sbase = ch["t0"][0]
                scnt = ch["nt"][0] + ch["nt"][1]
                s_sb = spool.tile(
                    [D, scnt * GW], MD, tag="S", name=f"S{l}_{sbase}"
                )
                nc.sync.dma_start(
                    s_sb[:], sflat_d[:, sbase * GW : (sbase + scnt) * GW]
                )
                ap = psum_agg.tile([D, GW], F32, tag="agg", name=f"agg{l}_{gv}")
                for i, (hh, off, gt) in enumerate(ch["tiles"]):
                    nc.tensor.matmul(
                        ap[:],
                        mb[hh][:, off, :],
                        s_sb[:, (gt - sbase) * GW : (gt - sbase + 1) * GW],
                        start=(i == 0),
                        stop=False,
                    )
                # self-loop terms: p_local windows against diag(dinv^2)
                for j in range(2):
                    wv = 2 * gv + j
                    nc.tensor.matmul(
                        ap[:, j * WIN : (j + 1) * WIN],
                        plocal[:, wv, :],
                        diag_sb[:, wv * WIN : (wv + 1) * WIN],
                        start=False,
                        stop=(j == 1),
                    )
                nc.scalar.activation(
                    HTn[:, gv * GW : (gv + 1) * GW],
                    ap[:],
                    AF.Relu,
                    bias=b_sb[l][:, 0:1],
                )
                if not last:
                    for j in range(2):
                        wv = 2 * gv + j
                        emit_pmm(HTn, l + 1, wv, agin_n, plocal_n)
                        if wv == nA - 1:
                            emit_ag(agin_n, pfull_n, 0, l + 1)

            pend = []
            for ci, ch in enumerate(plan.chunks):
                mA = gather_h(ch, 0, ci)
                pend.append((ch, mA, ci))
                if len(pend) > 1:
                    pch, pA, pci = pend.pop(0)
                    pB = gather_h(pch, 1, pci)
                    process(pch, {0: pA, 1: pB})
            while pend:
                pch, pA, pci = pend.pop(0)
                pB = gather_h(pch, 1, pci)
                process(pch, {0: pA, 1: pB})
            if not last:
                emit_ag(agin_n, pfull_n, 1, l + 1)
                agin, pfull, plocal = agin_n, pfull_n, plocal_n
            HT = HTn

        # ---- head: out = H3 @ lin_w + lin_b ----
        stage = pstage.tile([D, nwin], F32, tag="stage")
        for w in range(nwin):
            op = psum_p.tile([D, 1], F32, tag="op", name=f"op{w}", bufs=1)
            nc.tensor.matmul(
                op[:], HT[:, w * WIN : (w + 1) * WIN], linw_sb[:, :], start=True,
                stop=True,
            )
            nc.vector.tensor_scalar(
                stage[:, w : w + 1], op[:], linb_sb[:, 0:1], None, op0=ALU.add
            )
        tp = psum_p.tile([nwin, D], F32, tag="tp", bufs=1)
        nc.tensor.transpose(tp[:], stage[:], ident_sb[:])
        ov = pstage.tile([nwin, D], F32, tag="ov")
        nc.vector.tensor_copy(ov[:], tp[:])
        nc.sync.dma_start(out_d[:, :], ov[:])

    nc.compile()
    return nc


LAST = {}


def make_in_maps(inputs, per_core, cfg: Cfg):
    x = np.ascontiguousarray(np.asarray(inputs["x"], dtype=np.float32))
    Ws = [np.asarray(inputs[f"W{l}"], dtype=np.float32) for l in range(3)]
    bs = [np.asarray(inputs[f"b{l}"], dtype=np.float32) for l in range(3)]
    lin_w = np.asarray(inputs["lin_w"], dtype=np.float32)
    lin_b = np.asarray(inputs["lin_b"], dtype=np.float32)
    spad = cfg.spad
    ident = np.eye(D, dtype=np.float32)
    in_maps = []
    for c in range(NC):
        xs = x[c * cfg.shard : (c + 1) * cfg.shard]
        xT = np.zeros((D, spad), np.float16)
        xT[:, : cfg.shard] = xs.T.astype(np.float16)
        im = {
            "xT": xT,
            "lin_w": lin_w.astype(np.float32).reshape(D, 1),
            "lin_b": np.full((D, 1), float(lin_b.reshape(-1)[0]), np.float32),
            "ident": ident,
            "gidx": per_core[c]["gidx"],
            "sflat": per_core[c]["sflat"],
            "diag": per_core[c]["diag"],
        }
        for l in range(3):
            im[f"W{l}"] = Ws[l]
            im[f"b{l}"] = bs[l].reshape(D, 1)
        in_maps.append(im)
    return in_maps


def kernel(**inputs):
    cfg = Cfg()
    edge_index = np.asarray(inputs["edge_index"], dtype=np.int32)
    plan, per_core, _ = preprocess(edge_index, cfg)
    nc = build_program(plan, cfg)
    in_maps = make_in_maps(inputs, per_core, cfg)

    res = run_bass_kernel_spmd(nc, in_maps, core_ids=list(range(NC)))
    LAST["res"] = res
    out = np.zeros(cfg.n, np.float32)
    for c in range(NC):
        out[c * cfg.shard : (c + 1) * cfg.shard] = res.results[c]["out"].reshape(-1)[
            : cfg.shard
        ]
    return out
